# revision 11
# baseline (speedup 1.0000x reference)
"""Eval-mode ClassConditionalBatchNorm2d on 8 Trainium2 NeuronCores.

Math: for each sample b with label l:
    use_class = (alpha > 0) & (class_counts[l] >= 100)
    mean/var  = blend of (global, class[l]) stats if use_class else global
    out       = (x - mean) / sqrt(var + eps) * weight + bias

This folds to a per-(sample, channel) affine:  out = x * scale + shift with
    scale[b,c] = weight[c] / sqrt(var[b,c] + eps)
    shift[b,c] = bias[c] - mean[b,c] * scale[b,c]

The [B=64, C=256] scale/shift tables are tiny (64 KB) and computed on host;
the device kernel streams x (196 MiB) through SBUF applying one fused DVE
tensor_scalar (mult+add, per-partition scalars) per tile — memory-bound.

Sharding: pure data parallel over batch. Each of the 8 cores gets 8 samples
(x shard [8, 256, 56*56]) plus its own [128, 32] scale/shift table arranged
so that column 4*b + 2*h + {0,1} holds (scale, shift) for sample b, channel
half h, with channels on partitions.
"""

import numpy as np
from contextlib import ExitStack

B, C, H, W = 64, 256, 56, 56
HW = H * W
N_CORES = 8
BPC = B // N_CORES  # samples per core
N_HALF = C // 128   # channel halves (partition tiles)
EPS = 1e-5
MIN_COUNT = 100.0

_PROGRAM_CACHE = {}
LAST_RESULTS = None  # BassKernelResults of the most recent run (for profiling)


def _build_program(iters=1, bufs=6, dyn_loop=None, in_place=False,
                   fuse_halves=False, split=1):
    """Build + compile the single-core SPMD Bass program (cached).

    iters > 1 repeats the identical sweep back-to-back inside one NEFF;
    dyn_loop=N wraps the sweep in a hardware For loop of N trips. Both are
    used only by the benchmark harness to measure per-sweep cost.
    in_place applies the affine into the input tile (one pool, more bufs).
    fuse_halves loads/stores both channel halves of a sample in one DMA.
    split > 1 cuts each plane DMA into `split` free-dim chunks.
    """
    key = (iters, bufs, dyn_loop, in_place, fuse_halves, split)
    if key in _PROGRAM_CACHE:
        return _PROGRAM_CACHE[key]

    import concourse.tile as tile
    from concourse import bacc, mybir

    f32 = mybir.dt.float32
    nc = bacc.Bacc(
        "TRN2", target_bir_lowering=False, debug=False, num_devices=N_CORES
    )
    x_ap = nc.dram_tensor("x", [BPC, C, HW], f32, kind="ExternalInput").ap()
    tab_ap = nc.dram_tensor(
        "tables", [128, BPC * N_HALF * 2], f32, kind="ExternalInput"
    ).ap()
    out_ap = nc.dram_tensor("out", [BPC, C, HW], f32, kind="ExternalOutput").ap()

    with tile.TileContext(nc) as tc:
        with ExitStack() as ctx:
            tabp = ctx.enter_context(tc.tile_pool(name="tab", bufs=1))
            xp = ctx.enter_context(tc.tile_pool(name="xs", bufs=bufs))
            outp = ctx.enter_context(tc.tile_pool(name="os", bufs=bufs))

            tab = tabp.tile([128, BPC * N_HALF * 2], f32)
            nc.sync.dma_start(tab[:], tab_ap[:])

            def sweep():
                if fuse_halves:
                    for b in range(BPC):
                        t = xp.tile([128, N_HALF, HW], f32)
                        nc.sync.dma_start(
                            t[:], x_ap[b].rearrange("(h p) f -> p h f", h=N_HALF)
                        )
                        o = t if in_place else outp.tile([128, N_HALF, HW], f32)
                        for h in range(N_HALF):
                            r = N_HALF * b + h
                            nc.vector.tensor_scalar(
                                o[:, h, :],
                                t[:, h, :],
                                tab[:, 2 * r : 2 * r + 1],
                                tab[:, 2 * r + 1 : 2 * r + 2],
                                mybir.AluOpType.mult,
                                mybir.AluOpType.add,
                            )
                        nc.sync.dma_start(
                            out_ap[b].rearrange("(h p) f -> p h f", h=N_HALF), o[:]
                        )
                    return
                fw = HW // split
                for b in range(BPC):
                    for h in range(N_HALF):
                        for s in range(split):
                            r = N_HALF * b + h
                            t = xp.tile([128, fw], f32)
                            nc.sync.dma_start(
                                t[:],
                                x_ap[b, 128 * h : 128 * (h + 1),
                                     s * fw : (s + 1) * fw],
                            )
                            o = t if in_place else outp.tile([128, fw], f32)
                            nc.vector.tensor_scalar(
                                o[:],
                                t[:],
                                tab[:, 2 * r : 2 * r + 1],
                                tab[:, 2 * r + 1 : 2 * r + 2],
                                mybir.AluOpType.mult,
                                mybir.AluOpType.add,
                            )
                            nc.sync.dma_start(
                                out_ap[b, 128 * h : 128 * (h + 1),
                                       s * fw : (s + 1) * fw],
                                o[:],
                            )

            if dyn_loop is not None:
                with tc.For_i(0, dyn_loop, 1):
                    for _ in range(iters):
                        sweep()
            else:
                for _ in range(iters):
                    sweep()

    nc.compile()
    _PROGRAM_CACHE[key] = nc
    return nc


def _scale_shift(labels, weight, bias, global_mean, global_var,
                 class_mean, class_var, class_counts, alpha):
    """Per-sample affine tables [B, C], mirroring the reference's f32 branch
    selection exactly; the weight/sqrt fold is done in f64 for accuracy."""
    labels = np.asarray(labels).astype(np.int64).reshape(-1)
    a = np.float32(np.asarray(alpha).reshape(()))
    one_m_a = np.float32(1.0) - a

    use_class = (float(a) > 0.0) & (
        np.asarray(class_counts, np.float32)[labels] >= np.float32(MIN_COUNT)
    )  # [B]
    gm = np.asarray(global_mean, np.float32)
    gv = np.asarray(global_var, np.float32)
    blend_mean = one_m_a * gm[None, :] + a * np.asarray(class_mean, np.float32)[labels]
    blend_var = np.clip(
        one_m_a * gv[None, :] + a * np.asarray(class_var, np.float32)[labels],
        np.float32(EPS),
        None,
    )
    mean = np.where(use_class[:, None], blend_mean, gm[None, :])  # [B, C] f32
    var = np.where(use_class[:, None], blend_var, gv[None, :])

    scale64 = np.asarray(weight, np.float64)[None, :] / np.sqrt(
        var.astype(np.float64) + np.float64(EPS)
    )
    shift64 = np.asarray(bias, np.float64)[None, :] - mean.astype(np.float64) * scale64
    return scale64.astype(np.float32), shift64.astype(np.float32)


def kernel(x, labels, weight, bias, global_mean, global_var,
           class_mean, class_var, class_counts, alpha):
    global LAST_RESULTS
    from concourse.bass_utils import run_bass_kernel_spmd

    x = np.asarray(x, np.float32)
    scale, shift = _scale_shift(
        labels, weight, bias, global_mean, global_var,
        class_mean, class_var, class_counts, alpha,
    )

    nc = _build_program()

    in_maps = []
    for c in range(N_CORES):
        xs = x[c * BPC : (c + 1) * BPC].reshape(BPC, C, HW)
        sc = scale[c * BPC : (c + 1) * BPC].reshape(BPC, N_HALF, 128)
        sh = shift[c * BPC : (c + 1) * BPC].reshape(BPC, N_HALF, 128)
        st = np.stack([sc, sh], axis=-1)  # [b, h, p, 2]
        tab = np.ascontiguousarray(
            st.transpose(2, 0, 1, 3).reshape(128, BPC * N_HALF * 2)
        )  # col = 4b + 2h + k
        in_maps.append({"x": np.ascontiguousarray(xs), "tables": tab})

    res = run_bass_kernel_spmd(nc, in_maps, list(range(N_CORES)))
    LAST_RESULTS = res

    out = np.empty((B, C, H, W), np.float32)
    for c in range(N_CORES):
        out[c * BPC : (c + 1) * BPC] = res.results[c]["out"].reshape(BPC, C, H, W)
    return out


# revision 13
# speedup vs baseline: 1.1227x; 1.1227x over previous
"""Eval-mode ClassConditionalBatchNorm2d on 8 Trainium2 NeuronCores.

Math: for each sample b with label l:
    use_class = (alpha > 0) & (class_counts[l] >= 100)
    mean/var  = blend of (global, class[l]) stats if use_class else global
    out       = (x - mean) / sqrt(var + eps) * weight + bias

This folds to a per-(sample, channel) affine:  out = x * scale + shift with
    scale[b,c] = weight[c] / sqrt(var[b,c] + eps)
    shift[b,c] = bias[c] - mean[b,c] * scale[b,c]

The [B=64, C=256] scale/shift tables are tiny (64 KB) and computed on host;
the device kernel streams x (196 MiB) through SBUF applying one fused DVE
tensor_scalar (mult+add, per-partition scalars) per tile — memory-bound.

Sharding: pure data parallel over batch. Each of the 8 cores gets 8 samples
(x shard [8, 256, 56*56]) plus its own [128, 32] scale/shift table arranged
so that column 4*b + 2*h + {0,1} holds (scale, shift) for sample b, channel
half h, with channels on partitions.
"""

import numpy as np
from contextlib import ExitStack

B, C, H, W = 64, 256, 56, 56
HW = H * W
N_CORES = 8
BPC = B // N_CORES  # samples per core
N_HALF = C // 128   # channel halves (partition tiles)
EPS = 1e-5
MIN_COUNT = 100.0

_PROGRAM_CACHE = {}
LAST_RESULTS = None  # BassKernelResults of the most recent run (for profiling)


def _build_program(iters=1, bufs=6, dyn_loop=None, in_place=False,
                   fuse_halves=False, split=1):
    """Build + compile the single-core SPMD Bass program (cached).

    iters > 1 repeats the identical sweep back-to-back inside one NEFF;
    dyn_loop=N wraps the sweep in a hardware For loop of N trips. Both are
    used only by the benchmark harness to measure per-sweep cost.
    in_place applies the affine into the input tile (one pool, more bufs).
    fuse_halves=G >= 1 loads/stores G whole samples (both channel halves)
    per DMA. split > 1 cuts each plane DMA into `split` free-dim chunks.
    """
    fuse_halves = int(fuse_halves)
    key = (iters, bufs, dyn_loop, in_place, fuse_halves, split)
    if key in _PROGRAM_CACHE:
        return _PROGRAM_CACHE[key]

    import concourse.tile as tile
    from concourse import bacc, mybir

    f32 = mybir.dt.float32
    nc = bacc.Bacc(
        "TRN2", target_bir_lowering=False, debug=False, num_devices=N_CORES
    )
    x_ap = nc.dram_tensor("x", [BPC, C, HW], f32, kind="ExternalInput").ap()
    tab_ap = nc.dram_tensor(
        "tables", [128, BPC * N_HALF * 2], f32, kind="ExternalInput"
    ).ap()
    out_ap = nc.dram_tensor("out", [BPC, C, HW], f32, kind="ExternalOutput").ap()

    with tile.TileContext(nc) as tc:
        with ExitStack() as ctx:
            tabp = ctx.enter_context(tc.tile_pool(name="tab", bufs=1))
            xp = ctx.enter_context(tc.tile_pool(name="xs", bufs=bufs))
            outp = ctx.enter_context(tc.tile_pool(name="os", bufs=bufs))

            tab = tabp.tile([128, BPC * N_HALF * 2], f32)
            nc.sync.dma_start(tab[:], tab_ap[:])

            def sweep():
                if fuse_halves:
                    G = fuse_halves  # samples per tile
                    for b0 in range(0, BPC, G):
                        t = xp.tile([128, G * N_HALF, HW], f32)
                        src = x_ap[b0 : b0 + G].rearrange(
                            "g (h p) f -> p (g h) f", h=N_HALF
                        )
                        nc.sync.dma_start(t[:], src)
                        o = t if in_place else outp.tile([128, G * N_HALF, HW], f32)
                        for j in range(G * N_HALF):
                            r = N_HALF * b0 + j
                            nc.vector.tensor_scalar(
                                o[:, j, :],
                                t[:, j, :],
                                tab[:, 2 * r : 2 * r + 1],
                                tab[:, 2 * r + 1 : 2 * r + 2],
                                mybir.AluOpType.mult,
                                mybir.AluOpType.add,
                            )
                        dst = out_ap[b0 : b0 + G].rearrange(
                            "g (h p) f -> p (g h) f", h=N_HALF
                        )
                        nc.sync.dma_start(dst, o[:])
                    return
                fw = HW // split
                for b in range(BPC):
                    for h in range(N_HALF):
                        for s in range(split):
                            r = N_HALF * b + h
                            t = xp.tile([128, fw], f32)
                            nc.sync.dma_start(
                                t[:],
                                x_ap[b, 128 * h : 128 * (h + 1),
                                     s * fw : (s + 1) * fw],
                            )
                            o = t if in_place else outp.tile([128, fw], f32)
                            nc.vector.tensor_scalar(
                                o[:],
                                t[:],
                                tab[:, 2 * r : 2 * r + 1],
                                tab[:, 2 * r + 1 : 2 * r + 2],
                                mybir.AluOpType.mult,
                                mybir.AluOpType.add,
                            )
                            nc.sync.dma_start(
                                out_ap[b, 128 * h : 128 * (h + 1),
                                       s * fw : (s + 1) * fw],
                                o[:],
                            )

            if dyn_loop is not None:
                with tc.For_i(0, dyn_loop, 1):
                    for _ in range(iters):
                        sweep()
            else:
                for _ in range(iters):
                    sweep()

    nc.compile()
    _PROGRAM_CACHE[key] = nc
    return nc


def _scale_shift(labels, weight, bias, global_mean, global_var,
                 class_mean, class_var, class_counts, alpha):
    """Per-sample affine tables [B, C], mirroring the reference's f32 branch
    selection exactly; the weight/sqrt fold is done in f64 for accuracy."""
    labels = np.asarray(labels).astype(np.int64).reshape(-1)
    a = np.float32(np.asarray(alpha).reshape(()))
    one_m_a = np.float32(1.0) - a

    use_class = (float(a) > 0.0) & (
        np.asarray(class_counts, np.float32)[labels] >= np.float32(MIN_COUNT)
    )  # [B]
    gm = np.asarray(global_mean, np.float32)
    gv = np.asarray(global_var, np.float32)
    blend_mean = one_m_a * gm[None, :] + a * np.asarray(class_mean, np.float32)[labels]
    blend_var = np.clip(
        one_m_a * gv[None, :] + a * np.asarray(class_var, np.float32)[labels],
        np.float32(EPS),
        None,
    )
    mean = np.where(use_class[:, None], blend_mean, gm[None, :])  # [B, C] f32
    var = np.where(use_class[:, None], blend_var, gv[None, :])

    scale64 = np.asarray(weight, np.float64)[None, :] / np.sqrt(
        var.astype(np.float64) + np.float64(EPS)
    )
    shift64 = np.asarray(bias, np.float64)[None, :] - mean.astype(np.float64) * scale64
    return scale64.astype(np.float32), shift64.astype(np.float32)


def kernel(x, labels, weight, bias, global_mean, global_var,
           class_mean, class_var, class_counts, alpha):
    global LAST_RESULTS
    from concourse.bass_utils import run_bass_kernel_spmd

    x = np.asarray(x, np.float32)
    scale, shift = _scale_shift(
        labels, weight, bias, global_mean, global_var,
        class_mean, class_var, class_counts, alpha,
    )

    nc = _build_program()

    in_maps = []
    for c in range(N_CORES):
        xs = x[c * BPC : (c + 1) * BPC].reshape(BPC, C, HW)
        sc = scale[c * BPC : (c + 1) * BPC].reshape(BPC, N_HALF, 128)
        sh = shift[c * BPC : (c + 1) * BPC].reshape(BPC, N_HALF, 128)
        st = np.stack([sc, sh], axis=-1)  # [b, h, p, 2]
        tab = np.ascontiguousarray(
            st.transpose(2, 0, 1, 3).reshape(128, BPC * N_HALF * 2)
        )  # col = 4b + 2h + k
        in_maps.append({"x": np.ascontiguousarray(xs), "tables": tab})

    res = run_bass_kernel_spmd(nc, in_maps, list(range(N_CORES)))
    LAST_RESULTS = res

    out = np.empty((B, C, H, W), np.float32)
    for c in range(N_CORES):
        out[c * BPC : (c + 1) * BPC] = res.results[c]["out"].reshape(BPC, C, H, W)
    return out


# revision 15
# speedup vs baseline: 1.2135x; 1.0808x over previous
"""Eval-mode ClassConditionalBatchNorm2d on 8 Trainium2 NeuronCores.

Math: for each sample b with label l:
    use_class = (alpha > 0) & (class_counts[l] >= 100)
    mean/var  = blend of (global, class[l]) stats if use_class else global
    out       = (x - mean) / sqrt(var + eps) * weight + bias

This folds to a per-(sample, channel) affine:  out = x * scale + shift with
    scale[b,c] = weight[c] / sqrt(var[b,c] + eps)
    shift[b,c] = bias[c] - mean[b,c] * scale[b,c]

The [B=64, C=256] scale/shift tables are tiny (64 KB) and computed on host;
the device kernel streams x (196 MiB) through SBUF applying one fused DVE
tensor_scalar (mult+add, per-partition scalars) per channel-half — memory
bound, measured ~150-163 us/core vs the ~143.5 us HBM roofline
(2 x 25.7 MB per core at ~358 GB/s).

Sharding: pure data parallel over batch. Each of the 8 cores gets 8 samples
(x shard [8, 256, 56*56]) plus its own [128, 32] scale/shift table arranged
so that column 4*b + 2*h + {0,1} holds (scale, shift) for sample b, channel
half h, with channels on partitions. Tiles cover one whole sample
([128 partitions, 2 halves, 3136 spatial] = 3.2 MB) so each load/store is a
single large DMA that fans across all 16 SDMA ports; bufs=3 double-buffers
load/compute/store (2 pools x 3 x 3.2 MB = 19.2 MB SBUF).
"""

import numpy as np
from contextlib import ExitStack

B, C, H, W = 64, 256, 56, 56
HW = H * W
N_CORES = 8
BPC = B // N_CORES  # samples per core
N_HALF = C // 128   # channel halves (partition tiles)
EPS = 1e-5
MIN_COUNT = 100.0

_PROGRAM_CACHE = {}
LAST_RESULTS = None  # BassKernelResults of the most recent run (for profiling)


def _build_program(iters=1, bufs=6, dyn_loop=None, in_place=False,
                   fuse_halves=False, split=1):
    """Build + compile the single-core SPMD Bass program (cached).

    iters > 1 repeats the identical sweep back-to-back inside one NEFF;
    dyn_loop=N wraps the sweep in a hardware For loop of N trips. Both are
    used only by the benchmark harness to measure per-sweep cost.
    in_place applies the affine into the input tile (one pool, more bufs).
    fuse_halves=G >= 1 loads/stores G whole samples (both channel halves)
    per DMA. split > 1 cuts each plane DMA into `split` free-dim chunks.
    """
    fuse_halves = int(fuse_halves)
    key = (iters, bufs, dyn_loop, in_place, fuse_halves, split)
    if key in _PROGRAM_CACHE:
        return _PROGRAM_CACHE[key]

    import concourse.tile as tile
    from concourse import bacc, mybir

    f32 = mybir.dt.float32
    nc = bacc.Bacc(
        "TRN2", target_bir_lowering=False, debug=False, num_devices=N_CORES
    )
    x_ap = nc.dram_tensor("x", [BPC, C, HW], f32, kind="ExternalInput").ap()
    tab_ap = nc.dram_tensor(
        "tables", [128, BPC * N_HALF * 2], f32, kind="ExternalInput"
    ).ap()
    out_ap = nc.dram_tensor("out", [BPC, C, HW], f32, kind="ExternalOutput").ap()

    with tile.TileContext(nc) as tc:
        with ExitStack() as ctx:
            tabp = ctx.enter_context(tc.tile_pool(name="tab", bufs=1))
            xp = ctx.enter_context(tc.tile_pool(name="xs", bufs=bufs))
            outp = ctx.enter_context(tc.tile_pool(name="os", bufs=bufs))

            tab = tabp.tile([128, BPC * N_HALF * 2], f32)
            nc.sync.dma_start(tab[:], tab_ap[:])

            def sweep():
                if fuse_halves:
                    G = fuse_halves  # samples per tile
                    for b0 in range(0, BPC, G):
                        t = xp.tile([128, G * N_HALF, HW], f32)
                        src = x_ap[b0 : b0 + G].rearrange(
                            "g (h p) f -> p (g h) f", h=N_HALF
                        )
                        nc.sync.dma_start(t[:], src)
                        o = t if in_place else outp.tile([128, G * N_HALF, HW], f32)
                        for j in range(G * N_HALF):
                            r = N_HALF * b0 + j
                            nc.vector.tensor_scalar(
                                o[:, j, :],
                                t[:, j, :],
                                tab[:, 2 * r : 2 * r + 1],
                                tab[:, 2 * r + 1 : 2 * r + 2],
                                mybir.AluOpType.mult,
                                mybir.AluOpType.add,
                            )
                        dst = out_ap[b0 : b0 + G].rearrange(
                            "g (h p) f -> p (g h) f", h=N_HALF
                        )
                        nc.sync.dma_start(dst, o[:])
                    return
                fw = HW // split
                for b in range(BPC):
                    for h in range(N_HALF):
                        for s in range(split):
                            r = N_HALF * b + h
                            t = xp.tile([128, fw], f32)
                            nc.sync.dma_start(
                                t[:],
                                x_ap[b, 128 * h : 128 * (h + 1),
                                     s * fw : (s + 1) * fw],
                            )
                            o = t if in_place else outp.tile([128, fw], f32)
                            nc.vector.tensor_scalar(
                                o[:],
                                t[:],
                                tab[:, 2 * r : 2 * r + 1],
                                tab[:, 2 * r + 1 : 2 * r + 2],
                                mybir.AluOpType.mult,
                                mybir.AluOpType.add,
                            )
                            nc.sync.dma_start(
                                out_ap[b, 128 * h : 128 * (h + 1),
                                       s * fw : (s + 1) * fw],
                                o[:],
                            )

            if dyn_loop is not None:
                with tc.For_i(0, dyn_loop, 1):
                    for _ in range(iters):
                        sweep()
            else:
                for _ in range(iters):
                    sweep()

    nc.compile()
    _PROGRAM_CACHE[key] = nc
    return nc


def _scale_shift(labels, weight, bias, global_mean, global_var,
                 class_mean, class_var, class_counts, alpha):
    """Per-sample affine tables [B, C], mirroring the reference's f32 branch
    selection exactly; the weight/sqrt fold is done in f64 for accuracy."""
    labels = np.asarray(labels).astype(np.int64).reshape(-1)
    a = np.float32(np.asarray(alpha).reshape(()))
    one_m_a = np.float32(1.0) - a

    use_class = (float(a) > 0.0) & (
        np.asarray(class_counts, np.float32)[labels] >= np.float32(MIN_COUNT)
    )  # [B]
    gm = np.asarray(global_mean, np.float32)
    gv = np.asarray(global_var, np.float32)
    blend_mean = one_m_a * gm[None, :] + a * np.asarray(class_mean, np.float32)[labels]
    blend_var = np.clip(
        one_m_a * gv[None, :] + a * np.asarray(class_var, np.float32)[labels],
        np.float32(EPS),
        None,
    )
    mean = np.where(use_class[:, None], blend_mean, gm[None, :])  # [B, C] f32
    var = np.where(use_class[:, None], blend_var, gv[None, :])

    scale64 = np.asarray(weight, np.float64)[None, :] / np.sqrt(
        var.astype(np.float64) + np.float64(EPS)
    )
    shift64 = np.asarray(bias, np.float64)[None, :] - mean.astype(np.float64) * scale64
    return scale64.astype(np.float32), shift64.astype(np.float32)


def kernel(x, labels, weight, bias, global_mean, global_var,
           class_mean, class_var, class_counts, alpha):
    global LAST_RESULTS
    from concourse.bass_utils import run_bass_kernel_spmd

    x = np.asarray(x, np.float32)
    scale, shift = _scale_shift(
        labels, weight, bias, global_mean, global_var,
        class_mean, class_var, class_counts, alpha,
    )

    nc = _build_program(fuse_halves=1, bufs=3)

    in_maps = []
    for c in range(N_CORES):
        xs = x[c * BPC : (c + 1) * BPC].reshape(BPC, C, HW)
        sc = scale[c * BPC : (c + 1) * BPC].reshape(BPC, N_HALF, 128)
        sh = shift[c * BPC : (c + 1) * BPC].reshape(BPC, N_HALF, 128)
        st = np.stack([sc, sh], axis=-1)  # [b, h, p, 2]
        tab = np.ascontiguousarray(
            st.transpose(2, 0, 1, 3).reshape(128, BPC * N_HALF * 2)
        )  # col = 4b + 2h + k
        in_maps.append({"x": np.ascontiguousarray(xs), "tables": tab})

    res = run_bass_kernel_spmd(nc, in_maps, list(range(N_CORES)))
    LAST_RESULTS = res

    out = np.empty((B, C, H, W), np.float32)
    for c in range(N_CORES):
        out[c * BPC : (c + 1) * BPC] = res.results[c]["out"].reshape(BPC, C, H, W)
    return out


# revision 19
# speedup vs baseline: 1.2528x; 1.0324x over previous
"""Eval-mode ClassConditionalBatchNorm2d on 8 Trainium2 NeuronCores.

Math: for each sample b with label l:
    use_class = (alpha > 0) & (class_counts[l] >= 100)
    mean/var  = blend of (global, class[l]) stats if use_class else global
    out       = (x - mean) / sqrt(var + eps) * weight + bias

This folds to a per-(sample, channel) affine:  out = x * scale + shift with
    scale[b,c] = weight[c] / sqrt(var[b,c] + eps)
    shift[b,c] = bias[c] - mean[b,c] * scale[b,c]

The [B=64, C=256] scale/shift tables are tiny (64 KB) and computed on host;
the device kernel streams x (196 MiB) through SBUF applying one fused DVE
tensor_scalar (mult+add, per-partition scalars) per channel-half — memory
bound, measured ~150-163 us/core vs the ~143.5 us HBM roofline
(2 x 25.7 MB per core at ~358 GB/s).

Sharding: pure data parallel over batch. Each of the 8 cores gets 8 samples
(x shard [8, 256, 56*56]) plus its own [128, 32] scale/shift table arranged
so that column 4*b + 2*h + {0,1} holds (scale, shift) for sample b, channel
half h, with channels on partitions. Tiles cover one whole sample
([128 partitions, 2 halves, 3136 spatial] = 3.2 MB) so each load/store is a
single large DMA that fans across all 16 SDMA ports; bufs=3 double-buffers
load/compute/store (2 pools x 3 x 3.2 MB = 19.2 MB SBUF).
"""

import numpy as np
from contextlib import ExitStack

B, C, H, W = 64, 256, 56, 56
HW = H * W
N_CORES = 8
BPC = B // N_CORES  # samples per core
N_HALF = C // 128   # channel halves (partition tiles)
EPS = 1e-5
MIN_COUNT = 100.0

_PROGRAM_CACHE = {}
LAST_RESULTS = None  # BassKernelResults of the most recent run (for profiling)


def _build_program(iters=1, bufs=6, dyn_loop=None, in_place=False,
                   fuse_halves=False, split=1, obufs=None, store_swdge=False):
    """Build + compile the single-core SPMD Bass program (cached).

    iters > 1 repeats the identical sweep back-to-back inside one NEFF;
    dyn_loop=N wraps the sweep in a hardware For loop of N trips. Both are
    used only by the benchmark harness to measure per-sweep cost.
    in_place applies the affine into the input tile (one pool, more bufs).
    fuse_halves=G >= 1 loads/stores G whole samples (both channel halves)
    per DMA. split > 1 cuts each plane DMA into `split` free-dim chunks.
    """
    fuse_halves = int(fuse_halves)
    obufs = bufs if obufs is None else obufs
    key = (iters, bufs, dyn_loop, in_place, fuse_halves, split, obufs, store_swdge)
    if key in _PROGRAM_CACHE:
        return _PROGRAM_CACHE[key]

    import concourse.tile as tile
    from concourse import bacc, mybir

    f32 = mybir.dt.float32
    nc = bacc.Bacc(
        "TRN2", target_bir_lowering=False, debug=False, num_devices=N_CORES
    )
    x_ap = nc.dram_tensor("x", [BPC, C, HW], f32, kind="ExternalInput").ap()
    tab_ap = nc.dram_tensor(
        "tables", [128, BPC * N_HALF * 2], f32, kind="ExternalInput"
    ).ap()
    out_ap = nc.dram_tensor("out", [BPC, C, HW], f32, kind="ExternalOutput").ap()

    with tile.TileContext(nc) as tc:
        with ExitStack() as ctx:
            tabp = ctx.enter_context(tc.tile_pool(name="tab", bufs=1))
            xp = ctx.enter_context(tc.tile_pool(name="xs", bufs=bufs))
            outp = ctx.enter_context(tc.tile_pool(name="os", bufs=obufs))
            st_eng = nc.gpsimd if store_swdge else nc.sync

            tab = tabp.tile([128, BPC * N_HALF * 2], f32)
            nc.sync.dma_start(tab[:], tab_ap[:])

            def sweep():
                if fuse_halves:
                    G = fuse_halves  # samples per tile
                    for b0 in range(0, BPC, G):
                        t = xp.tile([128, G * N_HALF, HW], f32)
                        src = x_ap[b0 : b0 + G].rearrange(
                            "g (h p) f -> p (g h) f", h=N_HALF
                        )
                        nc.sync.dma_start(t[:], src)
                        o = t if in_place else outp.tile([128, G * N_HALF, HW], f32)
                        for j in range(G * N_HALF):
                            r = N_HALF * b0 + j
                            nc.vector.tensor_scalar(
                                o[:, j, :],
                                t[:, j, :],
                                tab[:, 2 * r : 2 * r + 1],
                                tab[:, 2 * r + 1 : 2 * r + 2],
                                mybir.AluOpType.mult,
                                mybir.AluOpType.add,
                            )
                        dst = out_ap[b0 : b0 + G].rearrange(
                            "g (h p) f -> p (g h) f", h=N_HALF
                        )
                        st_eng.dma_start(dst, o[:])
                    return
                fw = HW // split
                for b in range(BPC):
                    for h in range(N_HALF):
                        for s in range(split):
                            r = N_HALF * b + h
                            t = xp.tile([128, fw], f32)
                            nc.sync.dma_start(
                                t[:],
                                x_ap[b, 128 * h : 128 * (h + 1),
                                     s * fw : (s + 1) * fw],
                            )
                            o = t if in_place else outp.tile([128, fw], f32)
                            nc.vector.tensor_scalar(
                                o[:],
                                t[:],
                                tab[:, 2 * r : 2 * r + 1],
                                tab[:, 2 * r + 1 : 2 * r + 2],
                                mybir.AluOpType.mult,
                                mybir.AluOpType.add,
                            )
                            nc.sync.dma_start(
                                out_ap[b, 128 * h : 128 * (h + 1),
                                       s * fw : (s + 1) * fw],
                                o[:],
                            )

            if dyn_loop is not None:
                with tc.For_i(0, dyn_loop, 1):
                    for _ in range(iters):
                        sweep()
            else:
                for _ in range(iters):
                    sweep()

    nc.compile()
    _PROGRAM_CACHE[key] = nc
    return nc


def _scale_shift(labels, weight, bias, global_mean, global_var,
                 class_mean, class_var, class_counts, alpha):
    """Per-sample affine tables [B, C], mirroring the reference's f32 branch
    selection exactly; the weight/sqrt fold is done in f64 for accuracy."""
    labels = np.asarray(labels).astype(np.int64).reshape(-1)
    a = np.float32(np.asarray(alpha).reshape(()))
    one_m_a = np.float32(1.0) - a

    use_class = (float(a) > 0.0) & (
        np.asarray(class_counts, np.float32)[labels] >= np.float32(MIN_COUNT)
    )  # [B]
    gm = np.asarray(global_mean, np.float32)
    gv = np.asarray(global_var, np.float32)
    blend_mean = one_m_a * gm[None, :] + a * np.asarray(class_mean, np.float32)[labels]
    blend_var = np.clip(
        one_m_a * gv[None, :] + a * np.asarray(class_var, np.float32)[labels],
        np.float32(EPS),
        None,
    )
    mean = np.where(use_class[:, None], blend_mean, gm[None, :])  # [B, C] f32
    var = np.where(use_class[:, None], blend_var, gv[None, :])

    scale64 = np.asarray(weight, np.float64)[None, :] / np.sqrt(
        var.astype(np.float64) + np.float64(EPS)
    )
    shift64 = np.asarray(bias, np.float64)[None, :] - mean.astype(np.float64) * scale64
    return scale64.astype(np.float32), shift64.astype(np.float32)


def kernel(x, labels, weight, bias, global_mean, global_var,
           class_mean, class_var, class_counts, alpha):
    global LAST_RESULTS
    from concourse.bass_utils import run_bass_kernel_spmd

    x = np.asarray(x, np.float32)
    scale, shift = _scale_shift(
        labels, weight, bias, global_mean, global_var,
        class_mean, class_var, class_counts, alpha,
    )

    nc = _build_program(fuse_halves=1, bufs=3)

    in_maps = []
    for c in range(N_CORES):
        xs = x[c * BPC : (c + 1) * BPC].reshape(BPC, C, HW)
        sc = scale[c * BPC : (c + 1) * BPC].reshape(BPC, N_HALF, 128)
        sh = shift[c * BPC : (c + 1) * BPC].reshape(BPC, N_HALF, 128)
        st = np.stack([sc, sh], axis=-1)  # [b, h, p, 2]
        tab = np.ascontiguousarray(
            st.transpose(2, 0, 1, 3).reshape(128, BPC * N_HALF * 2)
        )  # col = 4b + 2h + k
        in_maps.append({"x": np.ascontiguousarray(xs), "tables": tab})

    res = run_bass_kernel_spmd(nc, in_maps, list(range(N_CORES)))
    LAST_RESULTS = res

    out = np.empty((B, C, H, W), np.float32)
    for c in range(N_CORES):
        out[c * BPC : (c + 1) * BPC] = res.results[c]["out"].reshape(BPC, C, H, W)
    return out
